# revision 14
# baseline (speedup 1.0000x reference)
"""Location-sensitive attention (Tacotron-style) on 8 TRN2 NeuronCores.

Data-parallel over batch: B=64 -> 8 batches per core, params replicated,
no collectives. Per core and per batch:
  kT_proj[A,T] = Wk^T @ key^T           (PE, bf16, contraction E=512)
  loc[F,T]     = conv1d(awc)            (PE via host-side im2col, k=62)
  locT[A,T]    = Wloc^T @ relu_bn(loc)  (PE, accumulated into same PSUM)
  tanh_out     = tanh(kT+locT + q bias) (ACT, per-partition bias)
  e[1,T]       = v^T @ tanh_out         (PE)
  softmax      = exp(e+maskadd) / sum   (ACT exp with fused row-sum accum)
  ctx[1,E]     = w^T @ key              (PE, contraction T, natural layout)

key is read twice in bf16 (transposed layout for the projection,
natural for the context reduction) = same HBM bytes as one f32 read.
"""

import sys

if '/opt/trn_rl_repo' not in sys.path:
    sys.path.insert(0, '/opt/trn_rl_repo')

from contextlib import ExitStack

import ml_dtypes
import numpy as np

B, T, E, R, A, F, K = 64, 1024, 512, 1024, 128, 32, 31
PAD = K // 2
NCORES = 8
BPC = B // NCORES  # batches per core
CK = 2 * K         # im2col contraction (62)
BN_EPS = 1e-5
BF16 = ml_dtypes.bfloat16


def _install_drain_patch():
    """This container's walrus rejects >2 sync waits on one CTRL instruction;
    split the TileContext kernel-tail drain's waits into standalone wait_ge ops."""
    import concourse.tile as tile
    from concourse.tile import ScopedClock

    if getattr(tile.TileContext, '_drain_patched', False):
        return

    def _patched(self, tick_clock, wait_clock):
        nc = self.nc
        probe = nc.sync.drain()
        wait_clock.add_sem_waits(probe.ins, ScopedClock({None: tick_clock.global_clock}))
        si = probe.ins.sync_info
        waits = list(si.on_wait) if si and si.on_wait else []
        if len(waits) > 2:
            si.on_wait = []
            by_name = {h.name: h for h in self.sems.allocated().values()}
            for w in waits:
                nc.sync.wait_ge(by_name[w.ant_name], w.wait_value)
            nc.sync.drain()
        nc.all_engine_barrier()
        assert self.sems is not None
        popped = nc._tile_sem_poison_stack.pop()
        assert popped is self._sem_poison
        nc.clear_and_free_semaphores(list(self.sems.allocated().values()))
        nc.all_engine_barrier()

    tile.TileContext._drain_and_barrier = _patched
    tile.TileContext._drain_patched = True


def _split_excess_waits(nc, max_waits=1):
    """This walrus build rejects instructions with >2 sync waits. Move the
    excess onto NOPs inserted just before the offender on the same engine."""
    import bass_rust

    for f in nc.m.functions:
        for bb in f.blocks:
            insts = bb.instructions
            idx = 0
            while idx < len(insts):
                inst = insts[idx]
                si = inst.sync_info
                waits = list(si.on_wait) if si is not None and si.on_wait else []
                if len(waits) > max_waits:
                    si.on_wait = waits[-max_waits:]
                    excess = waits[:-max_waits]
                    eng = nc.engines[inst.engine]
                    for j in range(0, len(excess), max_waits):
                        nop = eng.nop().ins
                        for f2 in nc.m.functions:
                            for bb2 in f2.blocks:
                                try:
                                    bb2.instructions.remove(nop)
                                except ValueError:
                                    pass
                        nop.sync_info = bass_rust.SyncInfo(
                            on_wait=excess[j:j + max_waits], on_update=[])
                        insts.insert(idx, nop)
                        idx += 1
                idx += 1


def build_bass(dbg=False):
    import concourse.bass as bass
    import concourse.tile as tile
    from concourse import mybir

    _install_drain_patch()
    dt = mybir.dt
    AF = mybir.ActivationFunctionType
    ALU = mybir.AluOpType

    nc = bass.Bass(name="locattn")
    keyT_p = nc.declare_dram_parameter("keyT", [BPC, E, T], dt.bfloat16, isOutput=False)
    keyN_p = nc.declare_dram_parameter("keyN", [BPC, T, E], dt.bfloat16, isOutput=False)
    X_p = nc.declare_dram_parameter("X", [BPC, CK, T], dt.bfloat16, isOutput=False)
    qT_p = nc.declare_dram_parameter("qT", [R, BPC], dt.bfloat16, isOutput=False)
    mask_p = nc.declare_dram_parameter("maskadd", [BPC, T], dt.float32, isOutput=False)
    Wq_p = nc.declare_dram_parameter("Wq", [R, A], dt.bfloat16, isOutput=False)
    Wk_p = nc.declare_dram_parameter("Wk", [E, A], dt.bfloat16, isOutput=False)
    Wloc_p = nc.declare_dram_parameter("Wloc", [F, A], dt.bfloat16, isOutput=False)
    W2_p = nc.declare_dram_parameter("W2", [CK, F], dt.bfloat16, isOutput=False)
    v_p = nc.declare_dram_parameter("v", [A, 1], dt.bfloat16, isOutput=False)
    sbn_p = nc.declare_dram_parameter("sbn", [F, 1], dt.float32, isOutput=False)
    bbn_p = nc.declare_dram_parameter("bbn", [F, 1], dt.float32, isOutput=False)
    id8_p = nc.declare_dram_parameter("id8", [BPC, BPC], dt.bfloat16, isOutput=False)
    octx_p = nc.declare_dram_parameter("out_ctx", [BPC, E], dt.float32, isOutput=True)
    ow_p = nc.declare_dram_parameter("out_w", [BPC, T], dt.float32, isOutput=True)
    escr_d = nc.dram_tensor("escr_d", [BPC, T], dt.float32)
    cscr_d = nc.dram_tensor("cscr_d", [BPC, E], dt.float32)
    if dbg:
        dbg_p = {
            "dbg_kt0": nc.declare_dram_parameter(
                "dbg_kt0", [128, 4 * T], dt.bfloat16, isOutput=True),
            "dbg_kn0": nc.declare_dram_parameter(
                "dbg_kn0", [128, 8 * E], dt.bfloat16, isOutput=True),
            "dbg_tanh0": nc.declare_dram_parameter(
                "dbg_tanh0", [128, T], dt.bfloat16, isOutput=True),
            "dbg_locb0": nc.declare_dram_parameter(
                "dbg_locb0", [F, T], dt.bfloat16, isOutput=True),
            "dbg_qt": nc.declare_dram_parameter(
                "dbg_qt", [128, BPC], dt.float32, isOutput=True),
            "dbg_eall": nc.declare_dram_parameter(
                "dbg_eall", [BPC, T], dt.float32, isOutput=True),
            "dbg_em": nc.declare_dram_parameter(
                "dbg_em", [BPC, T], dt.float32, isOutput=True),
            "dbg_wexp": nc.declare_dram_parameter(
                "dbg_wexp", [BPC, T], dt.bfloat16, isOutput=True),
            "dbg_ssum": nc.declare_dram_parameter(
                "dbg_ssum", [BPC, 1], dt.float32, isOutput=True),
            "dbg_wt": nc.declare_dram_parameter(
                "dbg_wt", [128, 8 * BPC], dt.bfloat16, isOutput=True),
            "dbg_ctxsb": nc.declare_dram_parameter(
                "dbg_ctxsb", [BPC, E], dt.float32, isOutput=True),
        }

    NEC = E // 128   # E chunks (4)
    NTC = T // 128   # T chunks (8)
    NRC = R // 128   # R chunks (8)

    with tile.TileContext(nc) as tc:
        with ExitStack() as ctx:
            cpool = ctx.enter_context(tc.tile_pool(name="consts", bufs=1))
            ppool = ctx.enter_context(tc.tile_pool(name="persist", bufs=1))

            # ---- replicated params -> SBUF
            Wq_sb = cpool.tile([128, NRC * A], dt.bfloat16, tag="wq")
            nc.sync.dma_start(
                Wq_sb[:].rearrange("p (c a) -> p c a", c=NRC),
                Wq_p[:].rearrange("(c p) a -> p c a", p=128))
            Wk_sb = cpool.tile([128, NEC * A], dt.bfloat16, tag="wk")
            nc.sync.dma_start(
                Wk_sb[:].rearrange("p (c a) -> p c a", c=NEC),
                Wk_p[:].rearrange("(c p) a -> p c a", p=128))
            Wloc_sb = cpool.tile([F, A], dt.bfloat16, tag="wloc")
            nc.sync.dma_start(Wloc_sb[:], Wloc_p[:])
            W2_sb = cpool.tile([CK, F], dt.bfloat16, tag="w2")
            nc.sync.dma_start(W2_sb[:], W2_p[:])
            v_sb = cpool.tile([A, 1], dt.bfloat16, tag="v")
            nc.sync.dma_start(v_sb[:], v_p[:])
            sbn_sb = cpool.tile([F, 1], dt.float32, tag="sbn")
            nc.sync.dma_start(sbn_sb[:], sbn_p[:])
            bbn_sb = cpool.tile([F, 1], dt.float32, tag="bbn")
            nc.sync.dma_start(bbn_sb[:], bbn_p[:])
            id8_sb = cpool.tile([BPC, BPC], dt.bfloat16, tag="id8")
            nc.sync.dma_start(id8_sb[:], id8_p[:])
            qTin_sb = cpool.tile([128, NRC * BPC], dt.bfloat16, tag="qtin")
            nc.sync.dma_start(
                qTin_sb[:].rearrange("p (c b) -> p c b", c=NRC),
                qT_p[:].rearrange("(c p) b -> p c b", p=128))
            mask_sb = cpool.tile([BPC, T], dt.float32, tag="mask")
            nc.sync.dma_start(mask_sb[:], mask_p[:])

            # ---- persistent working tiles
            qT_sb = ppool.tile([128, BPC], dt.float32, tag="qt")
            e_scr = ppool.tile([1, BPC * T], dt.float32, tag="escr")
            c_scr = ppool.tile([1, BPC * E], dt.float32, tag="cscr")
            e_all = ppool.tile([BPC, T], dt.float32, tag="eall")
            e_m = ppool.tile([BPC, T], dt.float32, tag="em")
            w_exp = ppool.tile([BPC, T], dt.bfloat16, tag="wexp")
            w_out = ppool.tile([BPC, T], dt.float32, tag="wout")
            wT_sb = ppool.tile([128, NTC * BPC], dt.bfloat16, tag="wt")
            s_sum = ppool.tile([BPC, 1], dt.float32, tag="ssum")
            rs = ppool.tile([BPC, 1], dt.float32, tag="rs")
            ctx_sb = ppool.tile([BPC, E], dt.float32, tag="ctxsb")
            ctx_n = ppool.tile([BPC, E], dt.float32, tag="ctxn")

            # ---- phase 0: q projection (all local batches at once)
            with tc.tile_pool(name="psq", bufs=1, space="PSUM") as psq:
                pq = psq.tile([128, BPC], dt.float32)
                for rc in range(NRC):
                    nc.tensor.matmul(
                        pq[:], Wq_sb[:, rc * A:(rc + 1) * A],
                        qTin_sb[:, rc * BPC:(rc + 1) * BPC],
                        start=(rc == 0), stop=(rc == NRC - 1))
                nc.scalar.copy(qT_sb[:], pq[:])
            if dbg:
                nc.sync.dma_start(dbg_p["dbg_qt"][:], qT_sb[:])

            # ---- phase A: per-batch energies
            knpool = ctx.enter_context(tc.tile_pool(name="keyn", bufs=BPC))
            kN = []
            with ExitStack() as actx:
                ktpool = actx.enter_context(tc.tile_pool(name="keyt", bufs=2))
                xpool = actx.enter_context(tc.tile_pool(name="xin", bufs=2))
                locpool = actx.enter_context(tc.tile_pool(name="locf", bufs=2))
                tanhpool = actx.enter_context(tc.tile_pool(name="tanh", bufs=2))
                ploc_pool = actx.enter_context(
                    tc.tile_pool(name="ploc", bufs=1, space="PSUM"))
                pe_pool = actx.enter_context(
                    tc.tile_pool(name="pe", bufs=2, space="PSUM"))
                pen_pool = actx.enter_context(
                    tc.tile_pool(name="pen", bufs=1, space="PSUM"))

                for b in range(BPC):
                    kT = ktpool.tile([128, NEC * T], dt.bfloat16, tag="kt")
                    for ec in range(NEC):
                        nc.sync.dma_start(
                            kT[:, ec * T:(ec + 1) * T],
                            keyT_p[b, ec * 128:(ec + 1) * 128, :])
                    kn = knpool.tile([128, NTC * E], dt.bfloat16, tag="kn")
                    kN.append(kn)
                    for tc_i in range(NTC):
                        nc.sync.dma_start(
                            kn[:, tc_i * E:(tc_i + 1) * E],
                            keyN_p[b, tc_i * 128:(tc_i + 1) * 128, :])
                    if dbg and b == 0:
                        nc.sync.dma_start(dbg_p["dbg_kt0"][:], kT[:])
                        nc.sync.dma_start(dbg_p["dbg_kn0"][:], kn[:])
                    Xb = xpool.tile([CK, T], dt.bfloat16, tag="x")
                    nc.sync.dma_start(Xb[:], X_p[b])

                    # conv as matmul (k=62), relu, folded batchnorm affine
                    ploc = ploc_pool.tile([F, T], dt.float32, tag="ploc")
                    for h in range(2):
                        ns = slice(h * 512, (h + 1) * 512)
                        nc.tensor.matmul(ploc[:, ns], W2_sb[:], Xb[:, ns],
                                         start=True, stop=True)
                    locf = locpool.tile([F, T], dt.bfloat16, tag="locf")
                    nc.scalar.activation(locf[:], ploc[:], AF.Relu)
                    locb = locpool.tile([F, T], dt.bfloat16, tag="locb")
                    nc.vector.tensor_scalar(locb[:], locf[:], sbn_sb[:], bbn_sb[:],
                                            op0=ALU.mult, op1=ALU.add)

                    # (k + loc)^T accumulated in PSUM, then tanh(+q) -> bf16
                    if dbg and b == 0:
                        nc.sync.dma_start(dbg_p["dbg_locb0"][:], locb[:])
                    pe_ = pe_pool.tile([128, T], dt.float32, tag="pe")
                    for h in range(2):
                        ns = slice(h * 512, (h + 1) * 512)
                        nc.tensor.matmul(pe_[:, ns], Wloc_sb[:], locb[:, ns],
                                         start=True, stop=False)
                        for ec in range(NEC):
                            nc.tensor.matmul(
                                pe_[:, ns], Wk_sb[:, ec * A:(ec + 1) * A],
                                kT[:, ec * T + h * 512: ec * T + (h + 1) * 512],
                                start=False, stop=(ec == NEC - 1))
                    th = tanhpool.tile([128, T], dt.bfloat16, tag="th")
                    nc.scalar.activation(th[:], pe_[:], AF.Tanh,
                                         bias=qT_sb[:, b:b + 1])

                    if dbg and b == 0:
                        nc.sync.dma_start(dbg_p["dbg_tanh0"][:], th[:])
                    # energies e[1,T] = v^T @ tanh
                    pen = pen_pool.tile([1, T], dt.float32, tag="pen")
                    for h in range(2):
                        ns = slice(h * 512, (h + 1) * 512)
                        nc.tensor.matmul(pen[0:1, ns], v_sb[:], th[:, ns],
                                         start=True, stop=True)
                    if b % 2 == 0:
                        nc.scalar.copy(e_scr[0:1, b * T:(b + 1) * T], pen[0:1, :])
                    else:
                        nc.vector.tensor_copy(e_scr[0:1, b * T:(b + 1) * T],
                                              pen[0:1, :])
                nc.sync.dma_start(
                    escr_d[:].rearrange("b t -> (b t)").rearrange("(o k) -> o k", o=1),
                    e_scr[:])
                nc.sync.dma_start(e_all[:], escr_d[:])
                if dbg:
                    nc.sync.dma_start(dbg_p["dbg_eall"][:], e_all[:])

            # ---- phase B: masked softmax (no max-sub needed: |e| <= ~6)
            nc.vector.tensor_tensor(e_m[:], e_all[:], mask_sb[:], op=ALU.add)
            if dbg:
                nc.sync.dma_start(dbg_p["dbg_em"][:], e_m[:])
            nc.scalar.activation(w_exp[:], e_m[:], AF.Exp, accum_out=s_sum[:])
            if dbg:
                nc.sync.dma_start(dbg_p["dbg_wexp"][:], w_exp[:])
                nc.sync.dma_start(dbg_p["dbg_ssum"][:], s_sum[:])
            nc.vector.reciprocal(rs[:], s_sum[:])
            nc.vector.tensor_scalar(w_out[:], w_exp[:], rs[:], None, op0=ALU.mult)
            nc.sync.dma_start(ow_p[:], w_out[:])

            # transpose w_exp -> [T-part, batch] for the context matmul
            with tc.tile_pool(name="pwt", bufs=1, space="PSUM") as pwt_pool:
                pwt = pwt_pool.tile([128, NTC * BPC], dt.bfloat16)
                for tc_i in range(NTC):
                    nc.tensor.transpose(
                        pwt[:, tc_i * BPC:(tc_i + 1) * BPC],
                        w_exp[:, tc_i * 128:(tc_i + 1) * 128],
                        id8_sb[:])
                nc.scalar.copy(wT_sb[:], pwt[:])
            if dbg:
                nc.sync.dma_start(dbg_p["dbg_wt"][:], wT_sb[:])

            # ---- phase C: context = w^T @ key (contraction T)
            with tc.tile_pool(name="pctx", bufs=2, space="PSUM") as pctx_pool:
                for b in range(BPC):
                    pctx = pctx_pool.tile([1, E], dt.float32, tag="pctx")
                    for tc_i in range(NTC):
                        nc.tensor.matmul(
                            pctx[0:1, :],
                            wT_sb[:, tc_i * BPC + b: tc_i * BPC + b + 1],
                            kN[b][:, tc_i * E:(tc_i + 1) * E],
                            start=(tc_i == 0), stop=(tc_i == NTC - 1))
                    if b % 2 == 0:
                        nc.scalar.copy(c_scr[0:1, b * E:(b + 1) * E], pctx[0:1, :])
                    else:
                        nc.vector.tensor_copy(c_scr[0:1, b * E:(b + 1) * E],
                                              pctx[0:1, :])
                nc.sync.dma_start(
                    cscr_d[:].rearrange("b e -> (b e)").rearrange("(o k) -> o k", o=1),
                    c_scr[:])
                nc.sync.dma_start(ctx_sb[:], cscr_d[:])
                if dbg:
                    nc.sync.dma_start(dbg_p["dbg_ctxsb"][:], ctx_sb[:])
            nc.vector.tensor_scalar(ctx_n[:], ctx_sb[:], rs[:], None, op0=ALU.mult)
            nc.sync.dma_start(octx_p[:], ctx_n[:])

    _split_excess_waits(nc)
    return nc


def to_bf16(x):
    """Round-to-nearest-even f32 -> bf16, vectorized (ml_dtypes astype is slow)."""
    x = np.ascontiguousarray(x, dtype=np.float32)
    b = x.view(np.uint32)
    r = ((b + 0x7FFF + ((b >> 16) & 1)) >> 16).astype(np.uint16)
    return r.view(BF16).reshape(x.shape)


def prep_inputs(query, key, attention_weights_cat, mask, Wq, Wk, conv_w,
                bn_gamma, bn_beta, bn_mean, bn_var, Wloc, v):
    f32 = np.float32
    key = np.asarray(key, f32)
    keyN = to_bf16(key)                                   # [B, T, E]
    keyT = np.ascontiguousarray(keyN.transpose(0, 2, 1))  # [B, E, T]

    awc = np.asarray(attention_weights_cat, f32)
    awc_pad = np.zeros((B, 2, T + 2 * PAD), f32)
    awc_pad[:, :, PAD:PAD + T] = awc
    Xw = np.lib.stride_tricks.sliding_window_view(awc_pad, T, axis=2)
    X = to_bf16(Xw.reshape(B, CK, T))                     # [B, 62, T]

    qT = to_bf16(np.asarray(query, f32).reshape(B, R).T)  # [R, B]
    maskadd = np.where(np.asarray(mask), f32(-1e30), f32(0)).astype(f32)

    s = (np.asarray(bn_gamma, f32)
         / np.sqrt(np.asarray(bn_var, f32) + BN_EPS)).reshape(F, 1)
    b2 = (np.asarray(bn_beta, f32).reshape(F, 1)
          - np.asarray(bn_mean, f32).reshape(F, 1) * s)
    W2 = to_bf16(np.asarray(conv_w, f32).transpose(1, 2, 0).reshape(CK, F))

    shared = {
        "qT_full": qT, "Wq": to_bf16(Wq), "Wk": to_bf16(Wk),
        "Wloc": to_bf16(Wloc), "W2": W2,
        "v": to_bf16(np.asarray(v, f32).reshape(A, 1)),
        "sbn": np.ascontiguousarray(s), "bbn": np.ascontiguousarray(b2),
        "id8": np.eye(BPC, dtype=BF16),
    }
    in_maps = []
    for c in range(NCORES):
        sl = slice(c * BPC, (c + 1) * BPC)
        in_maps.append({
            "keyT": keyT[sl], "keyN": keyN[sl], "X": X[sl],
            "qT": np.ascontiguousarray(qT[:, sl]),
            "maskadd": maskadd[sl],
            "Wq": shared["Wq"], "Wk": shared["Wk"], "Wloc": shared["Wloc"],
            "W2": shared["W2"], "v": shared["v"],
            "sbn": shared["sbn"], "bbn": shared["bbn"], "id8": shared["id8"],
        })
    return in_maps


_NC_CACHE = []


def run_on_hw(in_maps, trace=False, **kw):
    import concourse.bass_utils as bass_utils
    bass_utils.upload_artifacts = lambda tmpdir: tmpdir  # no cloud in container
    if not _NC_CACHE:
        _NC_CACHE.append(build_bass())
    nc = _NC_CACHE[0]
    return bass_utils.run_bass_kernel_spmd(
        nc, in_maps, core_ids=list(range(NCORES)), trace=trace, **kw)


def kernel(**inputs):
    in_maps = prep_inputs(**inputs)
    res = run_on_hw(in_maps, trace=False)
    ctx = np.concatenate([np.asarray(r["out_ctx"]) for r in res.results], axis=0)
    w = np.concatenate([np.asarray(r["out_w"]) for r in res.results], axis=0)
    return ctx.astype(np.float32), w.astype(np.float32)
